# revision 1
# baseline (speedup 1.0000x reference)
"""Trainium2 Bass kernel for AlignmentContrastiveLoss (8-core SPMD).

Math: with conserved c_i = (cat_i < 3), key k_i = label_i + 512*graph_i
(k pushed out of range for non-conserved rows),

  pos_cnt    = 1/2 (sum_L n_L^2 - sum_k n_k^2)
  sum_valid_sims = 1/2 (||U||_F^2 - ||W||_F^2)
      U[L,:] = sum_{i: l_i=L, c_i} e_i   (e = row-normalized embeddings)
      W[k,:] = sum_{i: k_i=k, c_i} e_i
  pos_sum    = pos_cnt - sum_valid_sims

so the O(N^2) masked gram reduces to one-hot matmuls [keys x N] @ [N x D].
Sharding: rows of the embedding are data-parallel across the 8 cores
(normalization), the one-hot/W matmul is sharded by key range (1024 keys
per core), with an all-gather of normalized embeddings and an all-reduce
of U partials / counts / scalar sums.  Negative pairs are sharded 625 per
core and use dma_gather on the all-gathered rows.
"""

import os
import sys

import numpy as np

if "/opt/trn_rl_repo" not in sys.path:
    sys.path.insert(0, "/opt/trn_rl_repo")

# persistent jax/neuron compile cache: repeat invocations skip the NEFF build
os.environ.setdefault("JAX_COMPILATION_CACHE_DIR", "/tmp/jaxcache")
os.environ.setdefault("JAX_PERSISTENT_CACHE_MIN_COMPILE_TIME_SECS", "1")
os.environ.setdefault("JAX_PERSISTENT_CACHE_MIN_ENTRY_SIZE_BYTES", "0")

import concourse.mybir as mybir  # noqa: E402
import concourse.tile as tile  # noqa: E402
from concourse import bacc  # noqa: E402
from concourse.bass_utils import run_bass_kernel_spmd  # noqa: E402

# Problem constants (hardcoded per the self-contained-kernel contract).
N, D, S = 8192, 512, 5000
M = 8                 # cores
R = N // M            # 1024 rows per core
RT = R // 128         # 8 row tiles per shard
NT = N // 128         # 64 row tiles total
KEYS = N // M         # 1024 (label,graph) keys per core; key = l + 512*g
EW = 640              # all-gathered row width: 512 emb + 4 meta + pad
SP = S // M           # 625 pairs per core
SPP = 640             # padded pairs per core (multiple of 128)
NIW = SPP // 16       # 40: dma_gather idx free width

F32 = mybir.dt.float32
BF16 = mybir.dt.bfloat16
I32 = mybir.dt.int32
I16 = mybir.dt.int16
ALU = mybir.AluOpType
ACTF = mybir.ActivationFunctionType
AX = mybir.AxisListType

_PROGRAM_CACHE = {}


def build_program(debug_dumps=False):
    """Build + compile the (single) SPMD Bass program. Returns nc."""
    if "nc" in _PROGRAM_CACHE:
        return _PROGRAM_CACHE["nc"]

    nc = bacc.Bacc("TRN2", target_bir_lowering=False, debug=False, num_devices=M)

    emb_d = nc.dram_tensor("emb", [R, D], BF16, kind="ExternalInput")
    lfull_d = nc.dram_tensor("lfull", [128, NT], F32, kind="ExternalInput")
    gfull_d = nc.dram_tensor("gfull", [128, NT], F32, kind="ExternalInput")
    cfull_d = nc.dram_tensor("cfull", [128, NT], F32, kind="ExternalInput")
    koff_d = nc.dram_tensor("koff", [128, 1], F32, kind="ExternalInput")
    lsh_d = nc.dram_tensor("lsh", [128, RT], F32, kind="ExternalInput")
    gsh_d = nc.dram_tensor("gsh", [128, RT], F32, kind="ExternalInput")
    csh_d = nc.dram_tensor("csh", [128, RT], F32, kind="ExternalInput")
    i1_d = nc.dram_tensor("i1", [128, NIW], I16, kind="ExternalInput")
    i2_d = nc.dram_tensor("i2", [128, NIW], I16, kind="ExternalInput")
    out_d = nc.dram_tensor("out", [1, 1], F32, kind="ExternalOutput")
    if debug_dumps:
        dbg_g1 = nc.dram_tensor("dbg_g1", [128, SPP // 128, EW], F32, kind="ExternalOutput")
        dbg_g2 = nc.dram_tensor("dbg_g2", [128, SPP // 128, EW], F32, kind="ExternalOutput")
        dbg_sims = nc.dram_tensor("dbg_sims", [128, SPP // 128], F32, kind="ExternalOutput")
        dbg_vmask = nc.dram_tensor("dbg_vmask", [128, SPP // 128], F32, kind="ExternalOutput")
        dbg_cols3 = nc.dram_tensor("dbg_cols3", [128, 4], F32, kind="ExternalOutput")
        dbg_scrow = nc.dram_tensor("dbg_scrow", [1, 512], F32, kind="ExternalOutput")
        dbg_i1 = nc.dram_tensor("dbg_i1", [128, NIW], I16, kind="ExternalOutput")

    groups = [list(range(M))]

    with tile.TileContext(nc) as tc:
        with (
            tc.tile_pool(name="cst", bufs=1) as cst,
            tc.tile_pool(name="sb", bufs=2) as sb,
            tc.tile_pool(name="rhsp", bufs=3) as rhsp,
            tc.tile_pool(name="ohp", bufs=2) as ohp,
            tc.tile_pool(name="drp", bufs=1, space="DRAM") as drp,
        ):
            # ---- constants / metadata ----
            iota_t = cst.tile([128, KEYS], I16, name="iota_t")
            nc.gpsimd.iota(iota_t[:], pattern=[[1, KEYS]], base=0, channel_multiplier=0)
            ones_bf = cst.tile([128, 1], BF16, name="ones_bf")
            nc.vector.memset(ones_bf[:], 1.0)
            ones_f32 = cst.tile([128, 1], F32, name="ones_f32")
            nc.vector.memset(ones_f32[:], 1.0)

            lfull = cst.tile([128, NT], F32, name="lfull")
            gfull = cst.tile([128, NT], F32, name="gfull")
            cfull = cst.tile([128, NT], F32, name="cfull")
            koff = cst.tile([128, 1], F32, name="koff")
            lsh = cst.tile([128, RT], F32, name="lsh")
            gsh = cst.tile([128, RT], F32, name="gsh")
            csh = cst.tile([128, RT], F32, name="csh")
            nc.sync.dma_start(lfull[:], lfull_d[:, :])
            nc.sync.dma_start(gfull[:], gfull_d[:, :])
            nc.sync.dma_start(cfull[:], cfull_d[:, :])
            nc.sync.dma_start(koff[:], koff_d[:, :])
            nc.sync.dma_start(lsh[:], lsh_d[:, :])
            nc.sync.dma_start(gsh[:], gsh_d[:, :])
            nc.sync.dma_start(csh[:], csh_d[:, :])

            # k_rel[p, t] = l + 512*g + 16384*(cat>2) - 1024*core  for row 128t+p
            # (all f32; ints < 2^24 are exact)
            k_f = cst.tile([128, NT], F32, name="k_f")
            ncons = cst.tile([128, NT], F32, name="ncons")
            nc.vector.tensor_scalar(k_f[:], gfull[:], 512.0, None, ALU.mult)
            nc.vector.tensor_tensor(k_f[:], k_f[:], lfull[:], ALU.add)
            nc.vector.tensor_scalar(ncons[:], cfull[:], 2.5, None, ALU.is_gt)
            nc.vector.scalar_tensor_tensor(
                k_f[:], ncons[:], 16384.0, k_f[:], ALU.mult, ALU.add
            )
            k_rel = cst.tile([128, NT], F32, name="k_rel")
            nc.vector.tensor_scalar(k_rel[:], k_f[:], koff[:], None, ALU.subtract)

            # ---- phase A: normalize shard rows, pack emb+meta, bounce to DRAM ----
            # batched meta for all 8 shard tiles: l//128 = sum of thresholds
            lhi_b = cst.tile([128, RT], F32, name="lhi_b")
            llo_b = cst.tile([128, RT], F32, name="llo_b")
            cons_b = cst.tile([128, RT], F32, name="cons_b")
            tmp_b = cst.tile([128, RT], F32, name="tmp_b")
            nc.vector.tensor_scalar(lhi_b[:], lsh[:], 128.0, None, ALU.is_ge)
            nc.vector.tensor_scalar(tmp_b[:], lsh[:], 256.0, None, ALU.is_ge)
            nc.vector.tensor_tensor(lhi_b[:], lhi_b[:], tmp_b[:], ALU.add)
            nc.vector.tensor_scalar(tmp_b[:], lsh[:], 384.0, None, ALU.is_ge)
            nc.vector.tensor_tensor(lhi_b[:], lhi_b[:], tmp_b[:], ALU.add)
            nc.vector.scalar_tensor_tensor(
                llo_b[:], lhi_b[:], -128.0, lsh[:], ALU.mult, ALU.add
            )
            nc.vector.tensor_scalar(cons_b[:], csh[:], 2.5, None, ALU.is_lt)

            bounce = drp.tile([R, EW], BF16, name="bounce")
            for j in range(RT):
                e_t = sb.tile([128, D], BF16, name=f"e_{j}", tag="eload", bufs=4)
                nc.sync.dma_start(e_t[:], emb_d[j * 128 : (j + 1) * 128, :])
                sqs = sb.tile([128, 1], F32, name=f"sqs_{j}", tag="sqs", bufs=4)
                scr = sb.tile([128, D], BF16, name=f"scr_{j}", tag="scr", bufs=4)
                nc.scalar.activation(scr[:], e_t[:], ACTF.Square, accum_out=sqs[:])
                nrm = sb.tile([128, 1], F32, name=f"nrm_{j}", tag="nrm", bufs=4)
                nc.scalar.activation(nrm[:], sqs[:], ACTF.Sqrt)
                inv = sb.tile([128, 1], F32, name=f"inv_{j}", tag="inv", bufs=4)
                nc.vector.reciprocal(inv[:], nrm[:])

                eb = sb.tile([128, EW], BF16, name=f"eb_{j}", tag="eb", bufs=4)
                nc.scalar.activation(
                    eb[:, 0:D], e_t[:], ACTF.Copy, scale=inv[:]
                )
                # meta columns: 512=l%128, 513=l//128, 514=g, 515=(cat<3)
                nc.vector.tensor_copy(eb[:, D : D + 1], llo_b[:, j : j + 1])
                nc.vector.tensor_copy(eb[:, D + 1 : D + 2], lhi_b[:, j : j + 1])
                nc.vector.tensor_copy(eb[:, D + 2 : D + 3], gsh[:, j : j + 1])
                nc.vector.tensor_copy(eb[:, D + 3 : D + 4], cons_b[:, j : j + 1])
                nc.vector.memset(eb[:, D + 4 : EW], 0.0)
                nc.sync.dma_start(bounce[j * 128 : (j + 1) * 128, :], eb[:])

            # ---- all-gather normalized rows (+meta) across cores ----
            gath = drp.tile([N, EW], BF16, name="gath", addr_space="Shared")
            nc.gpsimd.collective_compute(
                "AllGather",
                ALU.bypass,
                replica_groups=groups,
                ins=[bounce.opt()],
                outs=[gath.opt()],
            )

            # ---- phase C: one-hot key matmuls, W accum in PSUM ----
            pspW_cm = tc.tile_pool(name="pspW", bufs=1, space="PSUM")
            pspW = pspW_cm.__enter__()
            pw = [
                pspW.tile([128, 512], F32, name=f"pw{m}", tag=f"pw{m}")
                for m in range(8)
            ]
            o_sum = cst.tile([128, KEYS], BF16, name="o_sum")
            nc.vector.memset(o_sum[:], 0.0)

            for tb in range(8):
                rhs_l = []
                oh_l = []
                for tl in range(8):
                    t = 8 * tb + tl
                    rhs_t = rhsp.tile(
                        [128, D], BF16, name=f"rhs_{t}", tag=f"rhs{tl}"
                    )
                    nc.sync.dma_start(
                        rhs_t[:], gath[t * 128 : (t + 1) * 128, 0:D]
                    )
                    o_t = ohp.tile(
                        [128, KEYS], BF16, name=f"oh_{t}", tag=f"oh{tl}"
                    )
                    nc.vector.tensor_scalar(
                        o_t[:], iota_t[:], k_rel[:, t : t + 1], None, ALU.is_equal
                    )
                    nc.vector.tensor_tensor(o_sum[:], o_sum[:], o_t[:], ALU.add)
                    rhs_l.append(rhs_t)
                    oh_l.append(o_t)
                for m in range(8):
                    for tl in range(8):
                        t = 8 * tb + tl
                        nc.tensor.matmul(
                            pw[m][:, :],
                            oh_l[tl][:, 128 * m : 128 * (m + 1)],
                            rhs_l[tl][:],
                            start=(t == 0),
                            stop=(t == NT - 1),
                        )

            # ---- phase D: evacuate W, counts, local scalars ----
            # U partial: labels block b = pw[b] + pw[b+4] (keys = g_rel*512 + l)
            u_sb = []
            for b in range(4):
                # HW: tensor_tensor may read only one PSUM operand - stage one
                wcp = sb.tile([128, 512], F32, name=f"wcp_{b}", tag="wcp")
                nc.scalar.activation(wcp[:], pw[b][:, :], ACTF.Copy)
                u_t = sb.tile([128, 512], F32, name=f"u_{b}", tag=f"usb{b}")
                nc.vector.tensor_tensor(u_t[:], wcp[:], pw[b + 4][:, :], ALU.add)
                u_sb.append(u_t)

            wsqc = sb.tile([128, 8], F32, name="wsqc")
            for m in range(8):
                wscr = sb.tile([128, 512], BF16, name=f"wscr_{m}", tag="wscr")
                nc.scalar.activation(
                    wscr[:], pw[m][:, :], ACTF.Square, accum_out=wsqc[:, m : m + 1]
                )

            pspW_cm.__exit__(None, None, None)

            # counts: n_k = ones^T @ o_sum  (o_sum = sum of one-hots, ints <= 64)
            psp2_cm = tc.tile_pool(name="psp2", bufs=1, space="PSUM")
            psp2 = psp2_cm.__enter__()
            psc = psp2.tile([1, 1024], F32, name="psc")
            nc.tensor.matmul(
                psc[0:1, 0:512], ones_bf[:], o_sum[:, 0:512], start=True, stop=True
            )
            nc.tensor.matmul(
                psc[0:1, 512:1024], ones_bf[:], o_sum[:, 512:1024],
                start=True, stop=True,
            )

            nl_sb = sb.tile([1, 512], F32, name="nl_sb")
            nlc = sb.tile([1, 512], F32, name="nlc")
            nc.scalar.activation(nlc[:], psc[0:1, 0:512], ACTF.Copy)
            nc.vector.tensor_tensor(
                nl_sb[:], nlc[:], psc[0:1, 512:1024], ALU.add
            )

            # ---- phase E: negative pairs ----
            i1_t = cst.tile([128, NIW], I16, name="i1_t")
            i2_t = cst.tile([128, NIW], I16, name="i2_t")
            nc.sync.dma_start(i1_t[:], i1_d[:, :])
            nc.sync.dma_start(i2_t[:], i2_d[:, :])
            g1 = sb.tile([128, SPP // 128, EW], BF16, name="g1")
            g2 = sb.tile([128, SPP // 128, EW], BF16, name="g2")
            nc.gpsimd.dma_gather(
                g1[:], gath[:, :], i1_t[:], num_idxs=SPP, num_idxs_reg=SPP,
                elem_size=EW,
            )
            nc.gpsimd.dma_gather(
                g2[:], gath[:, :], i2_t[:], num_idxs=SPP, num_idxs_reg=SPP,
                elem_size=EW,
            )
            npair = SPP // 128  # 5
            prod = sb.tile([128, npair, D], BF16, name="prod")
            nc.vector.tensor_tensor(
                prod[:], g1[:, :, 0:D], g2[:, :, 0:D], ALU.mult
            )
            sims = sb.tile([128, npair], F32, name="sims")
            nc.vector.tensor_reduce(sims[:], prod[:], axis=AX.X, op=ALU.add)
            pen = sb.tile([128, npair], F32, name="pen")
            nc.vector.tensor_scalar(pen[:], sims[:], 0.0, None, ALU.max)

            l1 = sb.tile([128, npair], F32, name="l1")
            l2 = sb.tile([128, npair], F32, name="l2")
            nc.vector.scalar_tensor_tensor(
                l1[:], g1[:, :, D + 1 : D + 2], 128.0, g1[:, :, D : D + 1],
                ALU.mult, ALU.add,
            )
            nc.vector.scalar_tensor_tensor(
                l2[:], g2[:, :, D + 1 : D + 2], 128.0, g2[:, :, D : D + 1],
                ALU.mult, ALU.add,
            )
            vmask = sb.tile([128, npair], F32, name="vmask")
            nc.vector.tensor_tensor(vmask[:], l1[:], l2[:], ALU.not_equal)
            gmask = sb.tile([128, npair], F32, name="gmask")
            nc.vector.tensor_tensor(
                gmask[:], g1[:, :, D + 2 : D + 3], g2[:, :, D + 2 : D + 3],
                ALU.not_equal,
            )
            nc.vector.tensor_tensor(vmask[:], vmask[:], gmask[:], ALU.mult)
            cmask = sb.tile([128, npair], F32, name="cmask")
            nc.vector.tensor_tensor(
                cmask[:], g1[:, :, D + 3 : D + 4], g2[:, :, D + 3 : D + 4],
                ALU.add,
            )
            nc.vector.tensor_scalar(cmask[:], cmask[:], 0.5, None, ALU.is_gt)
            nc.vector.tensor_tensor(vmask[:], vmask[:], cmask[:], ALU.mult)
            nc.vector.tensor_tensor(pen[:], pen[:], vmask[:], ALU.mult)

            if debug_dumps:
                nc.sync.dma_start(dbg_i1[:, :], i1_t[:])
                g1f = sb.tile([128, SPP // 128, EW], F32, name="g1f")
                nc.vector.tensor_copy(g1f[:], g1[:])
                nc.sync.dma_start(dbg_g1[:, :, :], g1f[:])
                g2f = sb.tile([128, SPP // 128, EW], F32, name="g2f")
                nc.vector.tensor_copy(g2f[:], g2[:])
                nc.sync.dma_start(dbg_g2[:, :, :], g2f[:])
                nc.sync.dma_start(dbg_sims[:, :], sims[:])
                nc.sync.dma_start(dbg_vmask[:, :], vmask[:])

            # cols: 0 = ||W||^2 partial, 1 = neg_sum, 2 = neg_cnt
            cols3 = sb.tile([128, 4], F32, name="cols3")
            nc.vector.memset(cols3[:], 0.0)
            nc.vector.tensor_reduce(
                cols3[:, 0:1], wsqc[:], axis=AX.X, op=ALU.add
            )
            nc.vector.tensor_reduce(cols3[:, 1:2], pen[:], axis=AX.X, op=ALU.add)
            nc.vector.tensor_reduce(
                cols3[:, 2:3], vmask[:], axis=AX.X, op=ALU.add
            )
            psum_s = psp2.tile([1, 8], F32, name="psum_s")
            nc.tensor.matmul(
                psum_s[0:1, 0:4], ones_f32[:], cols3[:], start=True, stop=True
            )

            sc_row = sb.tile([1, 512], F32, name="sc_row")
            nc.vector.memset(sc_row[:], 0.0)
            nc.vector.tensor_copy(sc_row[:, 0:3], psum_s[0:1, 0:3])
            scscr = sb.tile([1, 1024], BF16, name="scscr")
            nc.scalar.activation(
                scscr[:], psc[0:1, :], ACTF.Square, accum_out=sc_row[:, 3:4]
            )

            if debug_dumps:
                nc.sync.dma_start(dbg_cols3[:, :], cols3[:])
                nc.sync.dma_start(dbg_scrow[:, :], sc_row[:])

            # ---- phase F: pack partials, all-reduce ----
            arb = drp.tile([514, 512], F32, name="arb")
            for b in range(4):
                nc.sync.dma_start(arb[b * 128 : (b + 1) * 128, :], u_sb[b][:])
            nc.sync.dma_start(arb[512:513, :], nl_sb[:])
            nc.sync.dma_start(arb[513:514, :], sc_row[:])
            aro = drp.tile([514, 512], F32, name="aro", addr_space="Shared")
            nc.gpsimd.collective_compute(
                "AllReduce",
                ALU.add,
                replica_groups=groups,
                ins=[arb.opt()],
                outs=[aro.opt()],
            )

            # ---- phase G: final scalar ----
            uf = sb.tile([128, 4, 512], F32, name="uf")
            nc.sync.dma_start(
                uf[:], aro[0:512, :].rearrange("(b p) d -> p b d", p=128)
            )
            uscr = sb.tile([128, 4, 512], BF16, name="uscr")
            u2red = sb.tile([128, 1], F32, name="u2red")
            nc.scalar.activation(uscr[:], uf[:], ACTF.Square, accum_out=u2red[:])
            psum_u = psp2.tile([1, 8], F32, name="psum_u")
            nc.tensor.matmul(
                psum_u[0:1, 0:1], ones_f32[:], u2red[:], start=True, stop=True
            )

            nlf = sb.tile([1, 512], F32, name="nlf")
            nc.sync.dma_start(nlf[:], aro[512:513, :])
            nlscr = sb.tile([1, 512], BF16, name="nlscr")
            nl2 = sb.tile([1, 1], F32, name="nl2")
            nc.scalar.activation(nlscr[:], nlf[:], ACTF.Square, accum_out=nl2[:])
            scf = sb.tile([1, 512], F32, name="scf")
            nc.sync.dma_start(scf[:], aro[513:514, :])

            # pos_cnt = 0.5*(sum nL^2 - sum nk^2)
            pc = sb.tile([1, 1], F32, name="pc")
            nc.vector.tensor_tensor(pc[:], nl2[:], scf[:, 3:4], ALU.subtract)
            nc.vector.tensor_scalar(pc[:], pc[:], 0.5, None, ALU.mult)
            # pos_sumsim = 0.5*(||U||^2 - ||W||^2)
            ps_ = sb.tile([1, 1], F32, name="ps_")
            nc.vector.tensor_tensor(
                ps_[:], psum_u[0:1, 0:1], scf[:, 0:1], ALU.subtract
            )
            nc.vector.tensor_scalar(ps_[:], ps_[:], 0.5, None, ALU.mult)
            # pos_loss = (pos_cnt - pos_sumsim) / max(pos_cnt,1) * (pos_cnt>0)
            psum_t = sb.tile([1, 1], F32, name="psum_t")
            nc.vector.tensor_tensor(psum_t[:], pc[:], ps_[:], ALU.subtract)
            den = sb.tile([1, 1], F32, name="den")
            nc.vector.tensor_scalar(den[:], pc[:], 1.0, None, ALU.max)
            rec = sb.tile([1, 1], F32, name="rec")
            nc.vector.reciprocal(rec[:], den[:])
            msk = sb.tile([1, 1], F32, name="msk")
            nc.vector.tensor_scalar(msk[:], pc[:], 0.0, None, ALU.is_gt)
            ploss = sb.tile([1, 1], F32, name="ploss")
            nc.vector.scalar_tensor_tensor(
                ploss[:], psum_t[:], rec[:], msk[:], ALU.mult, ALU.mult
            )
            # neg_loss
            den2 = sb.tile([1, 1], F32, name="den2")
            nc.vector.tensor_scalar(den2[:], scf[:, 2:3], 1.0, None, ALU.max)
            rec2 = sb.tile([1, 1], F32, name="rec2")
            nc.vector.reciprocal(rec2[:], den2[:])
            msk2 = sb.tile([1, 1], F32, name="msk2")
            nc.vector.tensor_scalar(msk2[:], scf[:, 2:3], 0.0, None, ALU.is_gt)
            nloss = sb.tile([1, 1], F32, name="nloss")
            nc.vector.scalar_tensor_tensor(
                nloss[:], scf[:, 1:2], rec2[:], msk2[:], ALU.mult, ALU.mult
            )

            outv = sb.tile([1, 1], F32, name="outv")
            nc.vector.tensor_tensor(outv[:], ploss[:], nloss[:], ALU.add)
            nc.sync.dma_start(out_d[:, :], outv[:])
            psp2_cm.__exit__(None, None, None)

    nc.compile()
    _PROGRAM_CACHE["nc"] = nc
    return nc


def make_in_maps(embeddings, labels, graph_ids, categories, idx1, idx2):
    """Host-side sharding / layout marshaling. Returns per-core input dicts."""
    import ml_dtypes

    emb = np.ascontiguousarray(
        np.asarray(embeddings, dtype=np.float32).astype(ml_dtypes.bfloat16)
    )
    l = np.asarray(labels).astype(np.float32)
    g = np.asarray(graph_ids).astype(np.float32)
    c = np.asarray(categories).astype(np.float32)
    i1 = np.asarray(idx1).astype(np.int64)
    i2 = np.asarray(idx2).astype(np.int64)
    assert emb.shape == (N, D) and l.shape == (N,) and i1.shape == (S,)

    # transposed [128, NT] layout: [p, t] = arr[128*t + p]
    lfull = np.ascontiguousarray(l.reshape(NT, 128).T)
    gfull = np.ascontiguousarray(g.reshape(NT, 128).T)
    cfull = np.ascontiguousarray(c.reshape(NT, 128).T)

    def wrap_idx(ix):
        # idx i lives at [i % 16, i // 16], replicated across the 8 groups of
        # 16 partitions (one per gpsimd sub-core).
        p = np.zeros(SPP, np.int16)
        p[: len(ix)] = ix.astype(np.int16)
        return np.ascontiguousarray(np.tile(p.reshape(NIW, 16).T, (8, 1)))

    in_maps = []
    for core in range(M):
        sl = slice(core * R, (core + 1) * R)
        in_maps.append(
            {
                "emb": emb[sl],
                "lfull": lfull,
                "gfull": gfull,
                "cfull": cfull,
                "koff": np.full((128, 1), core * KEYS, np.float32),
                "lsh": np.ascontiguousarray(l[sl].reshape(RT, 128).T),
                "gsh": np.ascontiguousarray(g[sl].reshape(RT, 128).T),
                "csh": np.ascontiguousarray(c[sl].reshape(RT, 128).T),
                "i1": wrap_idx(i1[core * SP : (core + 1) * SP]),
                "i2": wrap_idx(i2[core * SP : (core + 1) * SP]),
            }
        )
    return in_maps


def kernel(embeddings, labels, graph_ids, categories, idx1, idx2):
    nc = build_program()
    in_maps = make_in_maps(embeddings, labels, graph_ids, categories, idx1, idx2)
    res = run_bass_kernel_spmd(nc, in_maps, list(range(M)))
    out = np.asarray(res.results[0]["out"], dtype=np.float32)
    return out.reshape(())



# revision 8
# speedup vs baseline: 3.3217x; 3.3217x over previous
"""Trainium2 Bass kernel for AlignmentContrastiveLoss (8-core SPMD).

Math: the valid positive-pair set is sparse (512 labels over 8192 rows
-> ~16k pairs), so the O(N^2) masked gram is never formed. With
conserved c_i = (cat_i < 3), key k_i = label_i + 512*graph_i:

  pos_cnt   = 1/2 (sum_L n_L^2 - sum_k n_k^2)          (host, exact ints)
  S_pos_sims = 1/2 (||U||_F^2 - n_cons) - S_samekey
      U[L,:] = sum_{i: l_i=L, c_i} e_i   (e = row-normalized embeddings)
      S_samekey = sum over same-key conserved pairs (i<j) of sim_ij
  pos_loss  = 1 - S_pos_sims / pos_cnt

Sharding: core m owns labels [64m, 64m+64) -- the host sends it the
conserved rows with those labels (label-sharded), so each core computes
a COMPLETE slice of U via a [128 x 64] one-hot matmul and squares it
locally: ||U||^2 = sum_m ||U_m||^2 with no cross terms. The ~1.1k
same-key pairs and the 5k negative pairs are explicit row-pair lists
gathered host-side (integer index marshaling); the device computes
their cosine sims with fused multiply-reduce ops. A single 128-byte
AllReduce combines [||U_m||^2, S_samekey, S_neg, 1/8] and every core
finishes the scalar loss formula with host-folded constants.
"""

import os
import sys

import numpy as np

if "/opt/trn_rl_repo" not in sys.path:
    sys.path.insert(0, "/opt/trn_rl_repo")

# persistent jax/neuron compile cache: repeat invocations skip the NEFF build
os.environ.setdefault("JAX_COMPILATION_CACHE_DIR", "/tmp/jaxcache")
os.environ.setdefault("JAX_PERSISTENT_CACHE_MIN_COMPILE_TIME_SECS", "1")
os.environ.setdefault("JAX_PERSISTENT_CACHE_MIN_ENTRY_SIZE_BYTES", "0")

import concourse.mybir as mybir  # noqa: E402
import concourse.tile as tile  # noqa: E402
from concourse import bacc  # noqa: E402
from concourse.bass_utils import run_bass_kernel_spmd  # noqa: E402

# Problem constants (hardcoded per the self-contained-kernel contract).
N, D, S = 8192, 512, 5000
M = 8                 # cores
LPC = 64              # labels per core
ET = 6                # label-shard tiles (capacity 768 rows >= max ~582)
ECAP = ET * 128       # 768
PT = 7                # pair tiles (capacity 896 >= 625 neg + ~144 same-key)
PCAP = PT * 128       # 896
SP = S // M           # 625 negative pairs per core

F32 = mybir.dt.float32
BF16 = mybir.dt.bfloat16
I16 = mybir.dt.int16
ALU = mybir.AluOpType
ACTF = mybir.ActivationFunctionType
AX = mybir.AxisListType

_PROGRAM_CACHE = {}


def build_program():
    """Build + compile the (single) SPMD Bass program. Returns nc."""
    if "nc" in _PROGRAM_CACHE:
        return _PROGRAM_CACHE["nc"]

    nc = bacc.Bacc("TRN2", target_bir_lowering=False, debug=False, num_devices=M)

    esh_d = nc.dram_tensor("esh", [ECAP, D], BF16, kind="ExternalInput")
    krel_d = nc.dram_tensor("krel", [128, ET], F32, kind="ExternalInput")
    p1_d = nc.dram_tensor("p1", [128, PT, D], BF16, kind="ExternalInput")
    p2_d = nc.dram_tensor("p2", [128, PT, D], BF16, kind="ExternalInput")
    mk_d = nc.dram_tensor("mk", [128, PT], F32, kind="ExternalInput")
    mn_d = nc.dram_tensor("mn", [128, PT], F32, kind="ExternalInput")
    cvec_d = nc.dram_tensor("cvec", [1, 16], F32, kind="ExternalInput")
    out_d = nc.dram_tensor("out", [1, 1], F32, kind="ExternalOutput")

    groups = [list(range(M))]

    with tile.TileContext(nc) as tc:
        with (
            tc.tile_pool(name="cst", bufs=1) as cst,
            tc.tile_pool(name="sb", bufs=2) as sb,
            tc.tile_pool(name="psp", bufs=1, space="PSUM") as psp,
            tc.tile_pool(name="drp", bufs=1, space="DRAM") as drp,
        ):
            # ---- constants / metadata ----
            iota64 = cst.tile([128, 128], I16, name="iota64")
            nc.gpsimd.iota(iota64[:], pattern=[[1, 128]], base=0, channel_multiplier=0)
            ones_f32 = cst.tile([128, 1], F32, name="ones_f32")
            nc.vector.memset(ones_f32[:], 1.0)
            krel = cst.tile([128, ET], F32, name="krel")
            nc.sync.dma_start(krel[:], krel_d[:, :])
            mk_t = cst.tile([128, PT], F32, name="mk_t")
            mn_t = cst.tile([128, PT], F32, name="mn_t")
            nc.sync.dma_start(mk_t[:], mk_d[:, :])
            nc.sync.dma_start(mn_t[:], mn_d[:, :])
            cvec = cst.tile([1, 16], F32, name="cvec")
            nc.sync.dma_start(cvec[:], cvec_d[:, :])

            # ---- phase U: normalize label shard, one-hot matmul -> U slice ----
            psU = psp.tile([128, D], F32, name="psU")
            for t in range(ET):
                e_t = sb.tile([128, D], BF16, name=f"e_{t}", tag="eload", bufs=3)
                nc.sync.dma_start(e_t[:], esh_d[t * 128 : (t + 1) * 128, :])
                sqs = sb.tile([128, 1], F32, name=f"sqs_{t}", tag="sqs", bufs=3)
                scr = sb.tile([128, D], BF16, name=f"scr_{t}", tag="scr", bufs=3)
                nc.scalar.activation(scr[:], e_t[:], ACTF.Square, accum_out=sqs[:])
                nrm = sb.tile([128, 1], F32, name=f"nrm_{t}", tag="nrm", bufs=3)
                nc.scalar.activation(nrm[:], sqs[:], ACTF.Sqrt)
                inv = sb.tile([128, 1], F32, name=f"inv_{t}", tag="inv", bufs=3)
                nc.vector.reciprocal(inv[:], nrm[:])
                en = sb.tile([128, D], BF16, name=f"en_{t}", tag="en", bufs=3)
                nc.scalar.activation(en[:], e_t[:], ACTF.Copy, scale=inv[:])
                oh = sb.tile([128, 128], BF16, name=f"oh_{t}", tag="oh", bufs=3)
                nc.vector.tensor_scalar(
                    oh[:], iota64[:], krel[:, t : t + 1], None, ALU.is_equal
                )
                nc.tensor.matmul(
                    psU[:, :], oh[:], en[:], start=(t == 0), stop=(t == ET - 1)
                )
            u2scr = sb.tile([128, D], BF16, name="u2scr")
            u2col = sb.tile([128, 1], F32, name="u2col")
            nc.scalar.activation(u2scr[:], psU[:, :], ACTF.Square, accum_out=u2col[:])

            # ---- phase P: pair dot products (negatives + same-key) ----
            dots = sb.tile([128, PT], F32, name="dots")
            s11 = sb.tile([128, PT], F32, name="s11")
            s22 = sb.tile([128, PT], F32, name="s22")
            for t in range(PT):
                a_t = sb.tile([128, D], BF16, name=f"a_{t}", tag="aload", bufs=4)
                b_t = sb.tile([128, D], BF16, name=f"b_{t}", tag="bload", bufs=4)
                nc.sync.dma_start(a_t[:], p1_d[:, t, :])
                nc.sync.dma_start(b_t[:], p2_d[:, t, :])
                pscr = sb.tile([128, D], BF16, name=f"pscr_{t}", tag="pscr", bufs=4)
                nc.vector.tensor_tensor(pscr[:], a_t[:], b_t[:], ALU.mult)
                nc.vector.tensor_reduce(
                    dots[:, t : t + 1], pscr[:], axis=AX.X, op=ALU.add
                )
                ascr = sb.tile([128, D], BF16, name=f"ascr_{t}", tag="ascr", bufs=4)
                nc.scalar.activation(
                    ascr[:], a_t[:], ACTF.Square, accum_out=s11[:, t : t + 1]
                )
                bscr = sb.tile([128, D], BF16, name=f"bscr_{t}", tag="bscr", bufs=4)
                nc.scalar.activation(
                    bscr[:], b_t[:], ACTF.Square, accum_out=s22[:, t : t + 1]
                )

            nrm2 = sb.tile([128, PT], F32, name="nrm2")
            nc.vector.tensor_tensor(nrm2[:], s11[:], s22[:], ALU.mult)
            nrms = sb.tile([128, PT], F32, name="nrms")
            nc.scalar.activation(nrms[:], nrm2[:], ACTF.Sqrt)
            rin = sb.tile([128, PT], F32, name="rin")
            nc.vector.reciprocal(rin[:], nrms[:])
            sim = sb.tile([128, PT], F32, name="sim")
            nc.vector.tensor_tensor(sim[:], dots[:], rin[:], ALU.mult)

            # cols: 0 = ||U_m||^2 (partitions 0..63), 1 = S_samekey,
            #       2 = S_neg, 3 = 1/8 (AllReduce -> exact 1.0)
            cols = sb.tile([128, 4], F32, name="cols")
            nc.vector.memset(cols[:], 0.0)
            nc.vector.memset(cols[:, 3:4], 0.125 / 128.0)
            nc.vector.tensor_copy(cols[:, 0:1], u2col[:])
            skscr = sb.tile([128, PT], F32, name="skscr")
            nc.vector.tensor_tensor(skscr[:], sim[:], mk_t[:], ALU.mult)
            nc.vector.tensor_reduce(cols[:, 1:2], skscr[:], axis=AX.X, op=ALU.add)
            pen = sb.tile([128, PT], F32, name="pen")
            nc.vector.tensor_scalar(pen[:], sim[:], 0.0, None, ALU.max)
            snscr = sb.tile([128, PT], F32, name="snscr")
            nc.vector.tensor_tensor(snscr[:], pen[:], mn_t[:], ALU.mult)
            nc.vector.tensor_reduce(cols[:, 2:3], snscr[:], axis=AX.X, op=ALU.add)

            # ---- reduce partials across partitions, then across cores ----
            psS = psp.tile([1, 4], F32, name="psS")
            nc.tensor.matmul(psS[0:1, :], ones_f32[:], cols[:], start=True, stop=True)
            stage = sb.tile([1, 512], F32, name="stage")
            nc.vector.memset(stage[:], 0.0)
            nc.scalar.activation(stage[:, 0:4], psS[0:1, :], ACTF.Copy)
            arb = drp.tile([1, 512], F32, name="arb")
            nc.sync.dma_start(arb[:, :], stage[:])
            aro = drp.tile([1, 512], F32, name="aro", addr_space="Shared")
            nc.gpsimd.collective_compute(
                "AllReduce",
                ALU.add,
                replica_groups=groups,
                ins=[arb.opt()],
                outs=[aro.opt()],
            )

            # ---- epilogue: loss = sum(aro[0:16] * cvec) ----
            ar_sb = sb.tile([1, 512], F32, name="ar_sb")
            nc.sync.dma_start(ar_sb[:], aro[:, :])
            prod = sb.tile([1, 16], F32, name="prod")
            nc.vector.tensor_tensor(prod[:], ar_sb[:, 0:16], cvec[:], ALU.mult)
            outv = sb.tile([1, 1], F32, name="outv")
            nc.vector.tensor_reduce(outv[:], prod[:], axis=AX.X, op=ALU.add)
            nc.sync.dma_start(out_d[:, :], outv[:])

    nc.compile()
    _PROGRAM_CACHE["nc"] = nc
    return nc


def make_in_maps(embeddings, labels, graph_ids, categories, idx1, idx2):
    """Host-side sharding / layout marshaling. Returns per-core input dicts."""
    import ml_dtypes

    e32 = np.asarray(embeddings, dtype=np.float32)
    l = np.asarray(labels).astype(np.int64)
    g = np.asarray(graph_ids).astype(np.int64)
    c = np.asarray(categories).astype(np.int64)
    i1 = np.asarray(idx1).astype(np.int64)
    i2 = np.asarray(idx2).astype(np.int64)
    assert e32.shape == (N, D) and l.shape == (N,) and i1.shape == (S,)

    cons = c < 3
    n_cons = int(cons.sum())
    key = l + 512 * g

    # exact pair counts (integer metadata)
    lab_cnt = np.bincount(l[cons], minlength=512).astype(np.int64)
    key_cnt = np.bincount(key[cons], minlength=512 * 16).astype(np.int64)
    pos_cnt = int(((lab_cnt**2).sum() - (key_cnt**2).sum()) // 2)

    # same-key conserved pairs (i<j): enumerate via key-sorted groups
    cidx = np.nonzero(cons)[0]
    order = np.argsort(key[cidx], kind="stable")
    sidx = cidx[order]
    skey = key[sidx]
    bounds = np.nonzero(np.diff(skey))[0] + 1
    sk1, sk2 = [], []
    for grp in np.split(sidx, bounds):
        n = len(grp)
        if n < 2:
            continue
        ii, jj = np.triu_indices(n, k=1)
        sk1.append(grp[ii])
        sk2.append(grp[jj])
    sk1 = np.concatenate(sk1) if sk1 else np.zeros(0, np.int64)
    sk2 = np.concatenate(sk2) if sk2 else np.zeros(0, np.int64)

    # negative-pair validity mask
    negmask = (
        (g[i1] != g[i2]) & (l[i1] != l[i2]) & (cons[i1] | cons[i2])
    ).astype(np.float32)
    neg_cnt = int(negmask.sum())

    # scalar constants folded into the epilogue dot product
    cv = np.zeros((1, 16), np.float32)
    if pos_cnt > 0:
        cv[0, 0] = -0.5 / pos_cnt          # * ||U||^2
        cv[0, 1] = 1.0 / pos_cnt           # * S_samekey
        cv[0, 3] = 1.0 + 0.5 * n_cons / pos_cnt  # * 1.0
    if neg_cnt > 0:
        cv[0, 2] = 1.0 / neg_cnt           # * S_neg

    e_bf = e32.astype(ml_dtypes.bfloat16)
    ones_row = np.ones(D, ml_dtypes.bfloat16)

    sk_chunks = np.array_split(np.arange(len(sk1)), M)
    in_maps = []
    for m in range(M):
        # label shard: conserved rows with label in [64m, 64m+64)
        sel = np.nonzero(cons & (l // LPC == m))[0]
        assert len(sel) <= ECAP, f"label shard overflow: {len(sel)} > {ECAP}"
        esh = np.tile(ones_row, (ECAP, 1))
        esh[: len(sel)] = e_bf[sel]
        krel = np.full(ECAP, 999.0, np.float32)
        krel[: len(sel)] = (l[sel] - LPC * m).astype(np.float32)
        krel = np.ascontiguousarray(krel.reshape(ET, 128).T)

        # pair slots: negatives first, then this core's same-key chunk
        ck = sk_chunks[m]
        npair = SP + len(ck)
        assert npair <= PCAP, f"pair overflow: {npair} > {PCAP}"
        r1 = np.tile(ones_row, (PCAP, 1))
        r2 = np.tile(ones_row, (PCAP, 1))
        mkv = np.zeros(PCAP, np.float32)
        mnv = np.zeros(PCAP, np.float32)
        r1[:SP] = e_bf[i1[m * SP : (m + 1) * SP]]
        r2[:SP] = e_bf[i2[m * SP : (m + 1) * SP]]
        mnv[:SP] = negmask[m * SP : (m + 1) * SP]
        r1[SP:npair] = e_bf[sk1[ck]]
        r2[SP:npair] = e_bf[sk2[ck]]
        mkv[SP:npair] = 1.0

        in_maps.append(
            {
                "esh": np.ascontiguousarray(esh),
                "krel": krel,
                "p1": np.ascontiguousarray(
                    r1.reshape(PT, 128, D).transpose(1, 0, 2)
                ),
                "p2": np.ascontiguousarray(
                    r2.reshape(PT, 128, D).transpose(1, 0, 2)
                ),
                "mk": np.ascontiguousarray(mkv.reshape(PT, 128).T),
                "mn": np.ascontiguousarray(mnv.reshape(PT, 128).T),
                "cvec": cv,
            }
        )
    return in_maps


def kernel(embeddings, labels, graph_ids, categories, idx1, idx2):
    nc = build_program()
    in_maps = make_in_maps(embeddings, labels, graph_ids, categories, idx1, idx2)
    res = run_bass_kernel_spmd(nc, in_maps, list(range(M)))
    out = np.asarray(res.results[0]["out"], dtype=np.float32)
    return out.reshape(())


# revision 12
# speedup vs baseline: 9.2631x; 2.7886x over previous
"""Trainium2 Bass kernel for AlignmentContrastiveLoss (8-core SPMD).

Math: the valid positive-pair set is sparse (512 labels over 8192 rows
-> ~16k pairs), so the O(N^2) masked gram is never formed. With
conserved c_i = (cat_i < 3), key k_i = label_i + 512*graph_i:

  pos_cnt   = 1/2 (sum_L n_L^2 - sum_k n_k^2)          (host, exact ints)
  S_pos_sims = 1/2 (||U||_F^2 - n_cons) - S_samekey
      U[L,:] = sum_{i: l_i=L, c_i} e_i   (e = row-normalized embeddings)
      S_samekey = sum over same-key conserved pairs (i<j) of sim_ij
  pos_loss  = 1 - S_pos_sims / pos_cnt

Sharding: the 512 labels are greedily balanced into 8 groups (~64
labels, ~526 conserved rows each); core m receives the conserved rows
of its label group and computes a COMPLETE slice of U via a one-hot
matmul (the 1/||row|| factors are folded into the one-hot weights), so
||U||^2 = sum_m ||U_m||^2 with no cross terms. The ~1.1k same-key pairs
and the ~3.6k VALID negative pairs are explicit row-pair lists gathered
host-side (integer index marshaling) and split evenly; the device
computes their cosine sims (multiplies on GpSimd/Vector, free-dim
reduces on Vector, square-accumulates on Scalar). Each core writes its
three partial sums [||U_m||^2, S_samekey_m, S_neg_m]; the host
gathers/unshards the 8 partial outputs into the scalar loss (a device
AllReduce costs ~70us of one-time comm bootstrap + barrier skew,
dominating the ~20us of real work).
"""

import os
import sys

import numpy as np

if "/opt/trn_rl_repo" not in sys.path:
    sys.path.insert(0, "/opt/trn_rl_repo")

# persistent jax/neuron compile cache: repeat invocations skip the NEFF build
os.environ.setdefault("JAX_COMPILATION_CACHE_DIR", "/tmp/jaxcache")
os.environ.setdefault("JAX_PERSISTENT_CACHE_MIN_COMPILE_TIME_SECS", "1")
os.environ.setdefault("JAX_PERSISTENT_CACHE_MIN_ENTRY_SIZE_BYTES", "0")

import concourse.mybir as mybir  # noqa: E402
import concourse.tile as tile  # noqa: E402
from concourse import bacc  # noqa: E402
from concourse.bass_utils import run_bass_kernel_spmd  # noqa: E402

# Problem constants (hardcoded per the self-contained-kernel contract).
N, D, S = 8192, 512, 5000
M = 8                 # cores
LPC = 64              # labels per core (balanced groups of 512/8)
ET = 5                # label-shard tiles (capacity 640 rows >= max ~530)
ECAP = ET * 128       # 640
PT = 5                # pair tiles (capacity 640 >= (valid negs + same-key)/8)
PCAP = PT * 128       # 640

F32 = mybir.dt.float32
BF16 = mybir.dt.bfloat16
I16 = mybir.dt.int16
ALU = mybir.AluOpType
ACTF = mybir.ActivationFunctionType
AX = mybir.AxisListType

_PROGRAM_CACHE = {}

# tensor_tensor_reduce (fused multiply+reduce) crashes the NRT worker on
# this platform — do NOT use it; emit tensor_tensor + tensor_reduce.


def build_program():
    """Build + compile the (single) SPMD Bass program. Returns nc."""
    if "nc" in _PROGRAM_CACHE:
        return _PROGRAM_CACHE["nc"]

    nc = bacc.Bacc("TRN2", target_bir_lowering=False, debug=False, num_devices=M)

    esh_d = nc.dram_tensor("esh", [128, ET, D], BF16, kind="ExternalInput")
    krel_d = nc.dram_tensor("krel", [128, ET], F32, kind="ExternalInput")
    p1_d = nc.dram_tensor("p1", [128, PT, D], BF16, kind="ExternalInput")
    p2_d = nc.dram_tensor("p2", [128, PT, D], BF16, kind="ExternalInput")
    mk_d = nc.dram_tensor("mk", [128, PT], F32, kind="ExternalInput")
    mn_d = nc.dram_tensor("mn", [128, PT], F32, kind="ExternalInput")
    out_d = nc.dram_tensor("out", [1, 4], F32, kind="ExternalOutput")

    with tile.TileContext(nc) as tc:
        with (
            tc.tile_pool(name="cst", bufs=1) as cst,
            tc.tile_pool(name="sb", bufs=2) as sb,
            tc.tile_pool(name="psp", bufs=1, space="PSUM") as psp,
        ):
            # ---- constants / metadata; esh load issued first ----
            esh = cst.tile([128, ET, D], BF16, name="esh")
            nc.sync.dma_start(esh[:], esh_d[:, :, :])
            iota128 = cst.tile([128, 128], I16, name="iota128")
            nc.gpsimd.iota(iota128[:], pattern=[[1, 128]], base=0, channel_multiplier=0)
            ones_f32 = cst.tile([128, 1], F32, name="ones_f32")
            nc.vector.memset(ones_f32[:], 1.0)
            krel = cst.tile([128, ET], F32, name="krel")
            nc.sync.dma_start(krel[:], krel_d[:, :])
            mk_t = cst.tile([128, PT], F32, name="mk_t")
            mn_t = cst.tile([128, PT], F32, name="mn_t")
            nc.sync.dma_start(mk_t[:], mk_d[:, :])
            nc.sync.dma_start(mn_t[:], mn_d[:, :])
            # preload the Sqrt activation table so the mid-pipeline Sqrt
            # doesn't stall on an ACT_TABLE_LOAD
            warm = cst.tile([1, 1], F32, name="warm")
            nc.scalar.activation(warm[:], ones_f32[0:1, 0:1], ACTF.Sqrt)

            # ---- phase U: one-hot matmul with 1/||row|| folded in ----
            sqs = sb.tile([128, ET], F32, name="sqs")
            for t in range(ET):
                scr = sb.tile([128, D], BF16, name=f"scr_{t}", tag="scr", bufs=3)
                nc.gpsimd.tensor_tensor(scr[:], esh[:, t, :], esh[:, t, :], ALU.mult)
                nc.vector.tensor_reduce(
                    sqs[:, t : t + 1], scr[:], axis=AX.X, op=ALU.add
                )
            nrmv = sb.tile([128, ET], F32, name="nrmv")
            nc.scalar.activation(nrmv[:], sqs[:], ACTF.Sqrt)
            invv = sb.tile([128, ET], F32, name="invv")
            nc.vector.reciprocal(invv[:], nrmv[:])

            # pair loads issued after the U-phase dependency chain so the
            # esh DMA gets the queues first
            p1 = cst.tile([128, PT, D], BF16, name="p1")
            nc.sync.dma_start(p1[:], p1_d[:, :, :])
            p2 = cst.tile([128, PT, D], BF16, name="p2")
            nc.sync.dma_start(p2[:], p2_d[:, :, :])

            psU = psp.tile([128, D], F32, name="psU")
            for t in range(ET):
                oh = sb.tile([128, 128], BF16, name=f"oh_{t}", tag="oh", bufs=3)
                nc.vector.tensor_scalar(
                    oh[:], iota128[:], krel[:, t : t + 1], invv[:, t : t + 1],
                    ALU.is_equal, ALU.mult,
                )
                nc.tensor.matmul(
                    psU[:, :], oh[:], esh[:, t, :], start=(t == 0), stop=(t == ET - 1)
                )
            u2scr = sb.tile([128, D], BF16, name="u2scr")
            u2col = sb.tile([128, 1], F32, name="u2col")
            nc.scalar.activation(u2scr[:], psU[:, :], ACTF.Square, accum_out=u2col[:])

            # ---- phase P: pair dot products (valid negatives + same-key) ----
            dots = sb.tile([128, PT], F32, name="dots")
            s11 = sb.tile([128, PT], F32, name="s11")
            s22 = sb.tile([128, PT], F32, name="s22")
            for t in range(PT):
                pscr = sb.tile([128, D], BF16, name=f"pscr_{t}", tag="pscr", bufs=3)
                nc.vector.tensor_tensor(pscr[:], p1[:, t, :], p2[:, t, :], ALU.mult)
                nc.vector.tensor_reduce(
                    dots[:, t : t + 1], pscr[:], axis=AX.X, op=ALU.add
                )
                ascr = sb.tile([128, D], BF16, name=f"ascr_{t}", tag="ascr", bufs=3)
                nc.scalar.activation(
                    ascr[:], p1[:, t, :], ACTF.Square, accum_out=s11[:, t : t + 1]
                )
                bscr = sb.tile([128, D], BF16, name=f"bscr_{t}", tag="bscr", bufs=3)
                nc.scalar.activation(
                    bscr[:], p2[:, t, :], ACTF.Square, accum_out=s22[:, t : t + 1]
                )

            # masked dot sums can start as soon as dots are done
            kdots = sb.tile([128, PT], F32, name="kdots")
            nc.vector.tensor_tensor(kdots[:], dots[:], mk_t[:], ALU.mult)
            mdots = sb.tile([128, PT], F32, name="mdots")
            nc.vector.scalar_tensor_tensor(
                mdots[:], dots[:], 0.0, mn_t[:], ALU.max, ALU.mult
            )

            nrm2 = sb.tile([128, PT], F32, name="nrm2")
            nc.vector.tensor_tensor(nrm2[:], s11[:], s22[:], ALU.mult)
            nrms = sb.tile([128, PT], F32, name="nrms")
            nc.scalar.activation(nrms[:], nrm2[:], ACTF.Sqrt)
            rin = sb.tile([128, PT], F32, name="rin")
            nc.vector.reciprocal(rin[:], nrms[:])

            # cols: 0 = ||U_m||^2, 1 = S_samekey, 2 = S_neg
            cols = sb.tile([128, 4], F32, name="cols")
            nc.vector.memset(cols[:], 0.0)
            nc.vector.tensor_copy(cols[:, 0:1], u2col[:])
            skv = sb.tile([128, PT], F32, name="skv")
            nc.vector.tensor_tensor(skv[:], kdots[:], rin[:], ALU.mult)
            nc.vector.tensor_reduce(cols[:, 1:2], skv[:], axis=AX.X, op=ALU.add)
            snv = sb.tile([128, PT], F32, name="snv")
            nc.vector.tensor_tensor(snv[:], mdots[:], rin[:], ALU.mult)
            nc.vector.tensor_reduce(cols[:, 2:3], snv[:], axis=AX.X, op=ALU.add)

            # ---- reduce partials across partitions; host combines cores ----
            psS = psp.tile([1, 4], F32, name="psS")
            nc.tensor.matmul(psS[0:1, :], ones_f32[:], cols[:], start=True, stop=True)
            stage = sb.tile([1, 4], F32, name="stage")
            nc.vector.tensor_copy(stage[:], psS[0:1, :])
            nc.sync.dma_start(out_d[:, :], stage[:])

    nc.compile()
    _PROGRAM_CACHE["nc"] = nc
    return nc


def make_in_maps(embeddings, labels, graph_ids, categories, idx1, idx2):
    """Host-side sharding / layout marshaling.

    Returns (per-core input dicts, cv) where cv holds the count-derived
    scalar constants for the final host-side combine.
    """
    import ml_dtypes

    e32 = np.asarray(embeddings, dtype=np.float32)
    l = np.asarray(labels).astype(np.int64)
    g = np.asarray(graph_ids).astype(np.int64)
    c = np.asarray(categories).astype(np.int64)
    i1 = np.asarray(idx1).astype(np.int64)
    i2 = np.asarray(idx2).astype(np.int64)
    assert e32.shape == (N, D) and l.shape == (N,) and i1.shape == (S,)

    cons = c < 3
    n_cons = int(cons.sum())
    key = l + 512 * g

    # exact pair counts (integer metadata)
    lab_cnt = np.bincount(l[cons], minlength=512).astype(np.int64)
    key_cnt = np.bincount(key[cons], minlength=512 * 16).astype(np.int64)
    pos_cnt = int(((lab_cnt**2).sum() - (key_cnt**2).sum()) // 2)

    # balanced label groups: greedily pack labels (largest count first)
    # into 8 groups of <=64 labels, minimizing the max row load
    group_of = np.full(512, -1, np.int64)
    slot_of = np.full(512, -1, np.int64)
    loads = np.zeros(M, np.int64)
    sizes = np.zeros(M, np.int64)
    for lab in np.argsort(-lab_cnt, kind="stable"):
        order = np.argsort(loads, kind="stable")
        for m in order:
            if sizes[m] < LPC:
                group_of[lab] = m
                slot_of[lab] = sizes[m]
                sizes[m] += 1
                loads[m] += lab_cnt[lab]
                break
    assert (group_of >= 0).all()

    # same-key conserved pairs (i<j): enumerate via key-sorted groups
    cidx = np.nonzero(cons)[0]
    order = np.argsort(key[cidx], kind="stable")
    sidx = cidx[order]
    skey = key[sidx]
    bounds = np.nonzero(np.diff(skey))[0] + 1
    sk1, sk2 = [], []
    for grp in np.split(sidx, bounds):
        n = len(grp)
        if n < 2:
            continue
        ii, jj = np.triu_indices(n, k=1)
        sk1.append(grp[ii])
        sk2.append(grp[jj])
    sk1 = np.concatenate(sk1) if sk1 else np.zeros(0, np.int64)
    sk2 = np.concatenate(sk2) if sk2 else np.zeros(0, np.int64)

    # negative pairs: keep only the valid ones (mask is host metadata)
    negmask = (g[i1] != g[i2]) & (l[i1] != l[i2]) & (cons[i1] | cons[i2])
    neg_cnt = int(negmask.sum())
    n1 = i1[negmask]
    n2 = i2[negmask]

    # unified pair list: [negatives | same-key], with is-neg flag
    a_all = np.concatenate([n1, sk1])
    b_all = np.concatenate([n2, sk2])
    isneg = np.zeros(len(a_all), np.float32)
    isneg[: len(n1)] = 1.0

    # scalar constants for the host-side combine:
    # loss = cv[3] + cv[0]*||U||^2 + cv[1]*S_samekey + cv[2]*S_neg
    cv = np.zeros(4, np.float64)
    if pos_cnt > 0:
        cv[0] = -0.5 / pos_cnt
        cv[1] = 1.0 / pos_cnt
        cv[3] = 1.0 + 0.5 * n_cons / pos_cnt
    if neg_cnt > 0:
        cv[2] = 1.0 / neg_cnt

    e_bf = e32.astype(ml_dtypes.bfloat16)
    ones_row = np.ones(D, ml_dtypes.bfloat16)

    pair_chunks = np.array_split(np.arange(len(a_all)), M)
    in_maps = []
    for m in range(M):
        # label shard: conserved rows whose label group is m
        sel = np.nonzero(cons & (group_of[l] == m))[0]
        assert len(sel) <= ECAP, f"label shard overflow: {len(sel)} > {ECAP}"
        esh = np.tile(ones_row, (ECAP, 1))
        esh[: len(sel)] = e_bf[sel]
        krel = np.full(ECAP, 999.0, np.float32)
        krel[: len(sel)] = slot_of[l[sel]].astype(np.float32)
        krel = np.ascontiguousarray(krel.reshape(ET, 128).T)

        ck = pair_chunks[m]
        npair = len(ck)
        assert npair <= PCAP, f"pair overflow: {npair} > {PCAP}"
        r1 = np.tile(ones_row, (PCAP, 1))
        r2 = np.tile(ones_row, (PCAP, 1))
        mkv = np.zeros(PCAP, np.float32)
        mnv = np.zeros(PCAP, np.float32)
        r1[:npair] = e_bf[a_all[ck]]
        r2[:npair] = e_bf[b_all[ck]]
        mnv[:npair] = isneg[ck]
        mkv[:npair] = 1.0 - isneg[ck]

        in_maps.append(
            {
                "esh": np.ascontiguousarray(
                    esh.reshape(ET, 128, D).transpose(1, 0, 2)
                ),
                "krel": krel,
                "p1": np.ascontiguousarray(
                    r1.reshape(PT, 128, D).transpose(1, 0, 2)
                ),
                "p2": np.ascontiguousarray(
                    r2.reshape(PT, 128, D).transpose(1, 0, 2)
                ),
                "mk": np.ascontiguousarray(mkv.reshape(PT, 128).T),
                "mn": np.ascontiguousarray(mnv.reshape(PT, 128).T),
            }
        )
    return in_maps, cv


def combine(res, cv):
    """Gather/unshard the per-core partial sums into the scalar loss."""
    parts = np.stack(
        [
            np.asarray(res.results[m]["out"], dtype=np.float64).reshape(-1)
            for m in range(M)
        ]
    )
    tot = parts.sum(axis=0)
    loss = cv[3] + cv[0] * tot[0] + cv[1] * tot[1] + cv[2] * tot[2]
    return np.float32(loss)


def kernel(embeddings, labels, graph_ids, categories, idx1, idx2):
    nc = build_program()
    in_maps, cv = make_in_maps(
        embeddings, labels, graph_ids, categories, idx1, idx2
    )
    res = run_bass_kernel_spmd(nc, in_maps, list(range(M)))
    return combine(res, cv).reshape(())
